# revision 34
# baseline (speedup 1.0000x reference)
"""Trainium2 Bass kernel for a dense transformer block.

Problem: nn_Block (B=8, N=1024, D=768, H=12, HID=3072), fp32.
Sharding: data-parallel over batch, one batch element per NeuronCore (8 cores).

Per-core program (all in one TileContext):
  LN1 (per-tile x, dual DMA queues) -> PE-transpose -> qkv in fp8e4
  DoubleRow (weights x16 host-side; 1/256 folded into the exp scale, V's
  x16 into wprojT/16); q,k feature-major, V token-major.
  attention per (i_chunk, head-pair): S=q@kT row-major (K=128 via
  zero-padded k), exp(+accum denom) on ACT, normalize (TS 4x) + additive
  bias (one full-tile TT 2x) on DVE, PE-transpose pairs into single bf16
  PSUM banks, clamp[0,1] on the PSUM->SBUF copy, P^T @ V -> O^T.
  proj: residual x rides the PE accumulation as a float32r identity
  matmul, bias as a K=1 rank-1 matmul, ACT copy-out (DVE stays free so
  LN2 overlaps proj). LN2 -> transpose, MLP bf16 hidden-chunked
  accumulating into x2, final bias add -> out.

Big SBUF tensors are split per consumer granularity (x 8 tiles, hT 6,
h2T/a1 halves): Tile dependency tracking is per-tile, and monolithic
tiles serialize consumers behind the last producer.
LN affine (w,b) is folded into the following weight matrices host-side.
Pool alloc/release is strict LIFO; qkv weight pools are allocated before
the x pool so their DMAs don't wait on LN1 (stack-address overlap).
"""

import numpy as np

import concourse.bass as bass
from concourse import bacc
import concourse.mybir as mybir
import concourse.tile as tile
from concourse.masks import make_identity

F32 = mybir.dt.float32
F32R = mybir.dt.float32r
BF16 = mybir.dt.bfloat16
FP8 = mybir.dt.float8e4
DR = mybir.MatmulPerfMode.DoubleRow
AF = mybir.ActivationFunctionType
ALU = mybir.AluOpType

B, N, D = 8, 1024, 768
HEADS, HD = 12, 64
HID = 4 * D
EPS = 1e-5
SCALE = HD ** -0.5

_CACHE = {}


def build_program(split_waits=True):
    key = ("nc", split_waits)
    if key in _CACHE:
        return _CACHE[key]

    nc = bacc.Bacc()

    x_h = nc.declare_dram_parameter("x", [N, D], F32, isOutput=False)
    amat_h = nc.declare_dram_parameter("amat", [N, N], F32, isOutput=False)
    wqkvT_h = nc.declare_dram_parameter("wqkvT", [D, 3 * D], FP8, isOutput=False)
    qkvb_h = nc.declare_dram_parameter("qkvb", [3 * D], F32, isOutput=False)
    wprojT_h = nc.declare_dram_parameter("wprojT", [D, D], BF16, isOutput=False)
    bproj_h = nc.declare_dram_parameter("bproj", [D], F32, isOutput=False)
    wfc1T_h = nc.declare_dram_parameter("wfc1T", [D, HID], BF16, isOutput=False)
    fc1b_h = nc.declare_dram_parameter("fc1b", [HID], F32, isOutput=False)
    wfc2T_h = nc.declare_dram_parameter("wfc2T", [HID, D], BF16, isOutput=False)
    bfc2_h = nc.declare_dram_parameter("bfc2", [D], F32, isOutput=False)
    cident_h = nc.declare_dram_parameter("cident", [128, 128], F32, isOutput=False)
    cones_h = nc.declare_dram_parameter("cones", [128], F32, isOutput=False)
    out_h = nc.declare_dram_parameter("out", [N, D], F32, isOutput=True)

    def bcast128(src_ap):
        # [n] dram vector -> [128, n] broadcast access pattern
        return bass.AP(
            tensor=src_ap.tensor,
            offset=src_ap.offset,
            ap=[[0, 128]] + [list(p) for p in src_ap.ap],
        )

    with tile.TileContext(nc) as tc:
        # ---- psum pools (live whole kernel; 4+2+2 = 8 banks) ----
        psum_mm = tc.alloc_tile_pool(name="psmm", bufs=2, space="PSUM")
        psum_tp = tc.alloc_tile_pool(name="pstp", bufs=2, space="PSUM")
        psum_pv = tc.alloc_tile_pool(name="pspv", bufs=2, space="PSUM")

        # ---- constants (live whole kernel) ----
        consts = tc.alloc_tile_pool(name="consts", bufs=1)
        ident = consts.tile([128, 128], F32, name="ident")
        make_identity(nc, ident)
        ident_bf = consts.tile([128, 128], BF16, name="ident_bf")
        make_identity(nc, ident_bf)
        eps_sb = consts.tile([128, 1], F32, name="eps_sb")
        nc.vector.memset(eps_sb, EPS)
        ident_r = consts.tile([128, 128], F32R, name="ident_r")
        ones_row = consts.tile([1, 128], F32R, name="ones_row")
        bproj_row = consts.tile([1, D], F32R, name="bproj_row")
        qkb_sb = consts.tile([128, 12], F32, name="qkb_sb")
        fc1b_sb = consts.tile([128, 24], F32, name="fc1b_sb")
        vbias_bc = consts.tile([128, D], F32, name="vbias_bc")
        bfc2_bc = consts.tile([128, D], F32, name="bfc2_bc")

        # ---- long-lived pools, allocated in lifetime order (LIFO stack) ----
        p_x2 = tc.alloc_tile_pool(name="p_x2", bufs=8)  # proj -> end
        x2ts = [p_x2.tile([128, D], F32R, name=f"x2_{i}", tag="x2") for i in range(8)]
        p_st = tc.alloc_tile_pool(name="p_st", bufs=4)  # LN scratch, reused by LN2
        p_OT = tc.alloc_tile_pool(name="p_OT", bufs=1)  # attention -> proj
        OT = p_OT.tile([128, 6, N], BF16, name="OT")
        p_qk = tc.alloc_tile_pool(name="p_qk", bufs=1)  # qkv -> attention
        qT = p_qk.tile([128, 6, N], BF16, name="qT")
        # kTe: even head rows (0:64) live, odd rows zero; kTo: the reverse.
        # Lets S matmuls run K=128 full-array: the zero half annihilates the
        # other head's q rows.
        kTe = p_qk.tile([128, 6, N], BF16, name="kTe")
        kTo = p_qk.tile([128, 6, N], BF16, name="kTo")
        p_V = tc.alloc_tile_pool(name="p_V", bufs=1)
        V_sb = p_V.tile([128, 8, D], BF16, name="V_sb")
        p_hT = tc.alloc_tile_pool(name="p_hT", bufs=6)  # LN1 -> qkv
        hTq = [
            [p_hT.tile([128, 2, 512], FP8, name=f"hT{dp}{h}") for h in range(2)]
            for dp in range(3)
        ]

        def layer_norm(src_of, dst_of, tiles=range(8)):
            # src_of/dst_of: it -> [128, D] view; dst = (src - mean) * rstd
            for it in tiles:
                src = src_of(it)
                stats = p_st.tile([128, 3, 6], F32, name="stats", tag="stats")
                for sg in range(3):
                    nc.vector.bn_stats(
                        out=stats[:, sg, :],
                        in_=src[:, sg * 256 : (sg + 1) * 256],
                    )
                mv = p_st.tile([128, 2], F32, name="mv", tag="mv")
                nc.vector.bn_aggr(out=mv, in_=stats)
                rstd = p_st.tile([128, 1], F32, name="rstd", tag="rstd")
                nc.scalar.activation(
                    out=rstd, in_=mv[:, 1:2], func=AF.Sqrt, bias=eps_sb
                )
                nc.vector.reciprocal(rstd, rstd)
                nc.vector.tensor_scalar(
                    dst_of(it),
                    src,
                    mv[:, 0:1],
                    rstd,
                    ALU.subtract,
                    ALU.mult,
                )

        def transpose_8xD_to_T(src_of, dst_of, ic4s=(0, 1)):
            # src_of: it -> [128, D] token-major view; dst_of(dt, ic4) -> the
            # [128, 512] feature-major destination slice
            for ic4 in ic4s:
                for dt in range(6):
                    ps = psum_tp.tile([128, 512], F32, name="psT", tag="tp")
                    for k in range(4):
                        nc.tensor.matmul(
                            ps[:, k * 128 : (k + 1) * 128],
                            lhsT=src_of(ic4 * 4 + k)[:, dt * 128 : (dt + 1) * 128],
                            rhs=ident,
                            is_transpose=True,
                            start=(k == 0),
                            stop=(k == 3),
                        )
                    nc.scalar.copy(dst_of(dt, ic4), ps)

        # ================= LN1 (in place over x) =================
        # qkv weight pools allocated before p_x: their SBUF space must not
        # overlap the x tiles, else the weight DMAs wait for LN1 to finish.
        p_wq = tc.alloc_tile_pool(name="p_wq", bufs=12)
        p_wv = tc.alloc_tile_pool(name="p_wv", bufs=2)
        # x arrives as 8 separate tiles (per-tile dependency tracking: LN of
        # tile i starts as soon as its own DMA lands) on two DMA queues,
        # emitted before the constant loads so LN1 is never queued behind
        # them.
        p_x = tc.alloc_tile_pool(name="p_x", bufs=8)
        xts = []
        for it in range(8):
            xt = p_x.tile([128, D], F32, name=f"x{it}", tag="x")
            xts.append(xt)
            q = nc.sync if it % 2 == 0 else nc.gpsimd
            q.dma_start(out=xt, in_=x_h[it * 128 : (it + 1) * 128, :])
        nc.sync.dma_start(out=ident_r, in_=cident_h[:, :].bitcast(F32R))
        nc.sync.dma_start(out=ones_row, in_=cones_h[:].unsqueeze(0).bitcast(F32R))
        nc.sync.dma_start(out=bproj_row, in_=bproj_h[:].unsqueeze(0).bitcast(F32R))
        nc.gpsimd.dma_start(
            out=qkb_sb, in_=qkvb_h[0 : 2 * D].rearrange("(t p) -> p t", p=128)
        )
        nc.gpsimd.dma_start(
            out=fc1b_sb, in_=fc1b_h[:].rearrange("(t p) -> p t", p=128)
        )
        ln1_tp = lambda i4: transpose_8xD_to_T(
            lambda it: xts[it],
            lambda dt, _i4: hTq[dt // 2][_i4][:, dt % 2, :],
            ic4s=(i4,),
        )
        layer_norm(lambda it: xts[it], lambda it: xts[it], tiles=range(0, 4))
        ln1_tp(0)
        layer_norm(lambda it: xts[it], lambda it: xts[it], tiles=range(4, 8))

        # ================= QKV =================
        # (note: LN1's second transpose half is emitted in the middle of the
        # q/k loop below, so the PE works on qkv tcn=0 while LN1 finishes)
        nc.gpsimd.memset(kTe[64:128, :, :], 0.0)
        nc.gpsimd.memset(kTo[0:64, :, :], 0.0)
        wqs = []
        for ft in range(12):
            wq = p_wq.tile([128, 6, 128], FP8, name="wq", tag="wq")
            wqs.append(wq)
            nc.gpsimd.dma_start(
                out=wq,
                in_=wqkvT_h[:, ft * 128 : (ft + 1) * 128].rearrange(
                    "(t p) f -> p t f", p=128
                ),
            )
        for tcn in range(2):
            if tcn == 1:
                # PE queue: LN1's ic4=1 transposes land after the tcn=0
                # matmuls (their hTq[..][0] inputs were ready much earlier)
                ln1_tp(1)
                p_x.release()
            for ft in range(12):
                wq = wqs[ft]
                ps = psum_mm.tile([128, 1024], F32, name="psq", tag="mm")
                for dp in range(3):
                    nc.tensor.matmul(
                        ps[:, 0:512],
                        lhsT=wq[:, 2 * dp : 2 * dp + 2, :],
                        rhs=hTq[dp][tcn],
                        start=(dp == 0),
                        stop=(dp == 2),
                        perf_mode=DR,
                    )
                sl = slice(tcn * 512, (tcn + 1) * 512)
                if ft < 6:
                    nc.scalar.activation(
                        out=qT[:, ft, sl], in_=ps[:, 0:512],
                        func=AF.Identity, bias=qkb_sb[:, ft : ft + 1],
                    )
                else:
                    col = ft - 6
                    nc.scalar.activation(
                        out=kTe[0:64, col, sl], in_=ps[0:64, 0:512],
                        func=AF.Identity, bias=qkb_sb[0:64, ft : ft + 1],
                    )
                    nc.vector.tensor_scalar(
                        kTo[64:128, col, sl],
                        ps[64:128, 0:512],
                        qkb_sb[64:128, ft : ft + 1],
                        None,
                        ALU.add,
                    )

        nc.gpsimd.dma_start(out=vbias_bc, in_=bcast128(qkvb_h[2 * D : 3 * D]))
        for f0, fw in ((0, 512), (512, 256)):
            wv = p_wv.tile([128, 6, 512], FP8, name="wv", tag="wv")
            nc.gpsimd.dma_start(
                out=wv[:, :, 0:fw],
                in_=wqkvT_h[:, 2 * D + f0 : 2 * D + f0 + fw].rearrange(
                    "(t p) f -> p t f", p=128
                ),
            )
            for it in range(8):
                ps = psum_mm.tile([128, 1024], F32, name="psv", tag="mm")
                for dp in range(3):
                    nc.tensor.matmul(
                        ps[:, 0:fw],
                        lhsT=hTq[dp][it // 4][
                            :, :, (it % 4) * 128 : (it % 4 + 1) * 128
                        ],
                        rhs=wv[:, 2 * dp : 2 * dp + 2, 0:fw],
                        start=(dp == 0),
                        stop=(dp == 2),
                        perf_mode=DR,
                    )
                nc.vector.tensor_add(
                    V_sb[:, it, f0 : f0 + fw], ps[:, 0:fw], vbias_bc[:, f0 : f0 + fw]
                )

        p_wv.release()
        p_wq.release()
        p_hT.release()

        # ================= attention =================
        p_wp = tc.alloc_tile_pool(name="p_wp", bufs=1)
        wproj = p_wp.tile([128, 6, D], BF16, name="wproj")
        nc.gpsimd.dma_start(
            out=wproj, in_=wprojT_h[:, :].rearrange("(t p) f -> p t f", p=128)
        )
        p_am = tc.alloc_tile_pool(name="p_am", bufs=2)
        p_e = tc.alloc_tile_pool(name="p_e", bufs=5)
        p_PT = tc.alloc_tile_pool(name="p_PT", bufs=4)
        p_dn = tc.alloc_tile_pool(name="p_dn", bufs=2)

        am_tiles = {}

        def load_am(ic):
            am = p_am.tile([128, 4, N], BF16, name="am", tag="am")
            nc.gpsimd.dma_start(
                out=am,
                in_=amat_h[ic * 512 : (ic + 1) * 512, :].rearrange(
                    "(t p) j -> p t j", p=128
                ),
            )
            am_tiles[ic] = am

        def stage_a(ic, hp):
            # S = q^T k row-major (K=128 via zero-padded k), exp + denom
            e0 = p_e.tile([128, 4, N], BF16, name="e0", tag="e")
            e1 = p_e.tile([128, 4, N], BF16, name="e1", tag="e")
            dens = p_dn.tile([128, 8], F32, name="dens", tag="dens")
            for it2 in range(4):
                isl = slice(ic * 512 + it2 * 128, ic * 512 + (it2 + 1) * 128)
                for e_h, kTz, c0 in ((e0, kTe, 0), (e1, kTo, 4)):
                    ps = psum_mm.tile([128, 1024], F32, name="psS", tag="mm")
                    for jc in range(2):
                        nc.tensor.matmul(
                            ps[:, jc * 512 : (jc + 1) * 512],
                            lhsT=qT[:, hp, isl],
                            rhs=kTz[:, hp, jc * 512 : (jc + 1) * 512],
                            start=True,
                            stop=True,
                        )
                    nc.scalar.activation(
                        out=e_h[:, it2, :],
                        in_=ps,
                        func=AF.Exp,
                        scale=SCALE / 256.0,
                        accum_out=dens[:, c0 + it2 : c0 + it2 + 1],
                    )
            return e0, e1, dens

        def stage_b(ic, hp, e0, e1, dens):
            h0, h1 = 2 * hp, 2 * hp + 1
            am = am_tiles[ic]
            rden = p_dn.tile([128, 8], F32, name="rden", tag="rden")
            nc.vector.reciprocal(rden, dens)
            for it2 in range(4):
                for e_h, c0 in ((e0, 0), (e1, 4)):
                    # 4x-mode tensor_scalar (per-it2: rden is a per-partition
                    # scalar that differs per q-tile)
                    nc.vector.tensor_scalar(
                        e_h[:, it2, :],
                        e_h[:, it2, :],
                        rden[:, c0 + it2 : c0 + it2 + 1],
                        None,
                        ALU.mult,
                    )
            for e_h in (e0, e1):
                # one 2x-mode tensor_tensor over the whole [128, 4096] tile
                # (amortizes the per-op overhead 4x)
                nc.vector.tensor_tensor(
                    out=e_h[:, :, :], in0=e_h[:, :, :], in1=am[:, :, :],
                    op=ALU.add,
                )
            PTs = []
            for ei, e_h in enumerate((e0, e1)):
                PT = p_PT.tile([128, 8, 512], BF16, name="PT", tag="PT")
                PTs.append(PT)
                for jp in range(4):  # two k-tiles per bf16 psum bank
                    ps = psum_tp.tile([128, 1024], BF16, name="psP", tag="tp")
                    for j2 in range(2):
                        jt = 2 * jp + j2
                        for k in range(4):
                            nc.tensor.matmul(
                                ps[:, j2 * 512 + k * 128 : j2 * 512 + (k + 1) * 128],
                                lhsT=e_h[:, k, jt * 128 : (jt + 1) * 128],
                                rhs=ident_bf,
                                is_transpose=True,
                                start=(j2 == 0 and k == 0),
                                stop=(j2 == 1 and k == 3),
                            )
                    if jp == 1 + 2 * ei:  # one pair per head on ACT
                        nc.scalar.activation(
                            out=PT[:, 2 * jp : 2 * jp + 2, :], in_=ps, func=AF.Relu
                        )
                    else:
                        nc.vector.tensor_scalar(
                            PT[:, 2 * jp : 2 * jp + 2, :], ps, 0.0, 1.0,
                            ALU.max, ALU.min,
                        )
            po = psum_pv.tile([128, 512], F32, name="po", tag="pv")
            for jt in range(8):
                nc.tensor.matmul(
                    po[0:64, :],
                    lhsT=V_sb[:, jt, h0 * 64 : (h0 + 1) * 64],
                    rhs=PTs[0][:, jt, :],
                    start=(jt == 0),
                    stop=(jt == 7),
                    tile_position=(0, 0),
                )
                nc.tensor.matmul(
                    po[64:128, :],
                    lhsT=V_sb[:, jt, h1 * 64 : (h1 + 1) * 64],
                    rhs=PTs[1][:, jt, :],
                    start=(jt == 0),
                    stop=(jt == 7),
                    tile_position=(0, 64),
                    skip_group_check=True,
                )
            nc.scalar.copy(OT[:, hp, ic * 512 : (ic + 1) * 512], po)

        steps = [(ic, hp) for ic in range(2) for hp in range(6)]
        load_am(0)
        pending = None
        for idx, (ic, hp) in enumerate(steps):
            if hp == 0 and ic + 1 < 2:
                load_am(ic + 1)
            staged = stage_a(ic, hp)
            if pending is not None:
                stage_b(*pending)
            pending = (ic, hp, *staged)
        stage_b(*pending)

        p_dn.release()
        p_PT.release()
        p_e.release()
        p_am.release()

        # ================= proj + residual -> x2 =================
        # residual x and bias ride the PE accumulation (identity / rank-1
        # matmuls); epilogue is a single ACT copy, keeping DVE free for LN2.
        for it in range(8):
            q = nc.sync if it % 2 == 0 else nc.gpsimd
            q.dma_start(
                out=x2ts[it],
                in_=x_h[it * 128 : (it + 1) * 128, :].bitcast(F32R),
            )
        for it in range(8):
            for f0, fw in ((0, 512), (512, 256)):
                ps = psum_mm.tile([128, 1024], F32, name="psp", tag="mm")
                for dt in range(6):
                    nc.tensor.matmul(
                        ps[:, 0:fw],
                        lhsT=(OT[:, dt, it * 128 : (it + 1) * 128]),
                        rhs=(wproj[:, dt, f0 : f0 + fw]),
                        start=(dt == 0),
                        stop=False,
                    )
                nc.tensor.matmul(
                    ps[:, 0:fw],
                    lhsT=ident_r,
                    rhs=x2ts[it][:, f0 : f0 + fw],
                    start=False,
                    stop=False,
                )
                nc.tensor.matmul(
                    ps[:, 0:fw],
                    lhsT=ones_row,
                    rhs=bproj_row[:, f0 : f0 + fw],
                    start=False,
                    stop=True,
                )
                nc.scalar.copy(x2ts[it][:, f0 : f0 + fw], ps[:, 0:fw])
        p_wp.release()
        p_V.release()
        p_qk.release()
        p_OT.release()

        # ================= LN2 =================
        p_h2T = tc.alloc_tile_pool(name="p_h2T", bufs=2)
        h2Th = [p_h2T.tile([128, 6, 512], BF16, name=f"h2T{h}") for h in range(2)]
        p_w1 = tc.alloc_tile_pool(name="p_w1", bufs=2)
        p_a1 = tc.alloc_tile_pool(name="p_a1", bufs=2)
        p_w2 = tc.alloc_tile_pool(name="p_w2", bufs=2)
        p_h2 = tc.alloc_tile_pool(name="p_h2", bufs=1)
        h2_sb = p_h2.tile([128, 8, D], F32, name="h2_sb")
        for ic4 in range(2):
            layer_norm(lambda it: x2ts[it], lambda it: h2_sb[:, it, :],
                       tiles=range(ic4 * 4, ic4 * 4 + 4))
            transpose_8xD_to_T(lambda it: h2_sb[:, it, :],
                               lambda dt, i4: h2Th[i4][:, dt, :],
                               ic4s=(ic4,))
        p_h2.release()

        # ============ MLP (hidden-chunked, accumulate into x2) ============
        for hc in range(4):
            w1 = p_w1.tile([128, 6, 6, 128], BF16, name="w1", tag="w1")
            nc.gpsimd.dma_start(
                out=w1,
                in_=wfc1T_h[:, hc * 768 : (hc + 1) * 768].rearrange(
                    "(t p) (s f) -> p t s f", p=128, f=128
                ),
            )
            a1h = [
                p_a1.tile([128, 6, 512], BF16, name=f"a1{h}", tag="a1")
                for h in range(2)
            ]
            for tcn in range(2):
                for hti in range(6):
                    ht = hc * 6 + hti
                    ps = psum_mm.tile([128, 1024], F32, name="ps1", tag="mm")
                    for dt in range(6):
                        nc.tensor.matmul(
                            ps[:, 0:512],
                            lhsT=(w1[:, dt, hti, :]),
                            rhs=(h2Th[tcn][:, dt, :]),
                            start=(dt == 0),
                            stop=(dt == 5),
                        )
                    nc.scalar.activation(
                        out=a1h[tcn][:, hti, :],
                        in_=ps[:, 0:512],
                        func=AF.Gelu,
                        bias=fc1b_sb[:, ht : ht + 1],
                    )
            for dc in range(3):
                w2 = p_w2.tile([128, 6, 256], BF16, name="w2", tag="w2")
                nc.gpsimd.dma_start(
                    out=w2,
                    in_=wfc2T_h[
                        hc * 768 : (hc + 1) * 768, dc * 256 : (dc + 1) * 256
                    ].rearrange("(t p) f -> p t f", p=128),
                )
                for it in range(8):
                    ps = psum_tp.tile([128, 512], F32, name="ps2", tag="tp")
                    for hti in range(6):
                        nc.tensor.matmul(
                            ps[:, 0:256],
                            lhsT=(
                                a1h[it // 4][
                                    :, hti, (it % 4) * 128 : (it % 4 + 1) * 128
                                ]
                            ),
                            rhs=(w2[:, hti, :]),
                            start=(hti == 0),
                            stop=(hti == 5),
                        )
                    sl = x2ts[it][:, dc * 256 : (dc + 1) * 256]
                    nc.vector.tensor_add(sl, ps[:, 0:256], sl)

        p_w2.release()
        p_a1.release()
        p_w1.release()
        p_h2T.release()

        # ================= final bias + store =================
        nc.gpsimd.dma_start(out=bfc2_bc, in_=bcast128(bfc2_h[:]))
        for it in range(8):
            nc.vector.tensor_add(x2ts[it], x2ts[it], bfc2_bc)
            nc.sync.dma_start(
                out=out_h[it * 128 : (it + 1) * 128, :].bitcast(F32R),
                in_=x2ts[it],
            )

        p_st.release()
        p_x2.release()
        consts.release()
        psum_pv.release()
        psum_tp.release()
        psum_mm.release()

    if split_waits:
        nc.compile()
    _CACHE[key] = nc
    return nc


def _split_matmul_waits(nc, max_mm_waits=1, chunk=4):
    """walrus's Matmult S3_LW struct supports very few semaphore waits; move
    a multi-wait matmul's waits onto PE NoOps inserted just before it (PE
    executes in order, so the waits still gate the matmul)."""
    n_split = 0
    for fn in nc.m.functions:
        for bb in fn.blocks:
            new = []
            for inst in bb.instructions:
                si = inst.sync_info
                if (
                    type(inst).__name__ == "InstMatmult"
                    and si is not None
                    and len(si.on_wait) > max_mm_waits
                ):
                    waits = list(si.on_wait)
                    for ci in range(0, len(waits), chunk):
                        nop = mybir.InstNoOp(
                            name=f"{inst.name}-w{ci}", ins=[], outs=[]
                        )
                        nop.engine = inst.engine
                        nop.sync_info = mybir.SyncInfo(
                            on_wait=waits[ci : ci + chunk], on_update=[]
                        )
                        new.append(nop)
                    inst.sync_info = mybir.SyncInfo(
                        on_wait=[], on_update=list(si.on_update)
                    )
                    n_split += 1
                new.append(inst)
            bb.instructions = new
    return n_split


def make_in_maps(inputs):
    f = lambda a: np.ascontiguousarray(np.asarray(a, dtype=np.float32))
    x = f(inputs["x"])
    amat = f(inputs["additional_matrix"])
    w_qkv = f(inputs["w_qkv"])
    ln1_w, ln1_b = f(inputs["ln1_w"]), f(inputs["ln1_b"])
    ln2_w, ln2_b = f(inputs["ln2_w"]), f(inputs["ln2_b"])
    w_fc1, b_fc1 = f(inputs["w_fc1"]), f(inputs["b_fc1"])

    import ml_dtypes

    bf = lambda a: np.ascontiguousarray(a.astype(ml_dtypes.bfloat16))
    import ml_dtypes as mld

    f8 = lambda a: np.ascontiguousarray(
        np.clip(a, -240.0, 240.0).astype(mld.float8_e4m3)
    )
    # qkv weights/bias are scaled x16 (dodges fp8e4 subnormals); q,k carry
    # x16 each so exp uses scale/256; V's x16 is folded into wprojT (/16).
    shared = {
        "wqkvT": f8(16.0 * ln1_w[:, None] * w_qkv.T),
        "qkvb": np.ascontiguousarray(16.0 * (ln1_b @ w_qkv.T)),
        "wprojT": bf(f(inputs["w_proj"]).T / 16.0),
        "bproj": f(inputs["b_proj"]),
        "wfc1T": bf(ln2_w[:, None] * w_fc1.T),
        "fc1b": np.ascontiguousarray(b_fc1 + ln2_b @ w_fc1.T),
        "wfc2T": bf(f(inputs["w_fc2"]).T),
        "bfc2": f(inputs["b_fc2"]),
        "cident": np.eye(128, dtype=np.float32),
        "cones": np.ones(128, dtype=np.float32),
    }
    return [
        {"x": np.ascontiguousarray(x[b]), "amat": np.ascontiguousarray(amat[b, 0]), **shared}
        for b in range(B)
    ]


def kernel(**inputs) -> np.ndarray:
    from concourse.bass_utils import run_bass_kernel_spmd

    nc = build_program()
    in_maps = make_in_maps(inputs)
    res = run_bass_kernel_spmd(nc, in_maps, list(range(B)))
    return np.stack([res.results[b]["out"] for b in range(B)]).astype(np.float32)



# revision 35
# speedup vs baseline: 1.0041x; 1.0041x over previous
"""Trainium2 Bass kernel for a dense transformer block.

Problem: nn_Block (B=8, N=1024, D=768, H=12, HID=3072), fp32.
Sharding: data-parallel over batch, one batch element per NeuronCore (8 cores).

Per-core program (all in one TileContext):
  LN1 (per-tile x, dual DMA queues) -> PE-transpose -> qkv in fp8e4
  DoubleRow (weights x16 host-side; 1/256 folded into the exp scale, V's
  x16 into wprojT/16); q,k feature-major, V token-major.
  attention per (i_chunk, head-pair): S=q@kT row-major (K=128 via
  zero-padded k), exp(+accum denom) on ACT, normalize (TS 4x) + additive
  bias (one full-tile TT 2x) on DVE, PE-transpose pairs into single bf16
  PSUM banks, clamp[0,1] on the PSUM->SBUF copy, P^T @ V -> O^T.
  proj: residual x rides the PE accumulation as a float32r identity
  matmul, bias as a K=1 rank-1 matmul, ACT copy-out (DVE stays free so
  LN2 overlaps proj). LN2 -> transpose, MLP bf16 hidden-chunked
  accumulating into x2, final bias add -> out.

Big SBUF tensors are split per consumer granularity (x 8 tiles, hT 6,
h2T/a1 halves): Tile dependency tracking is per-tile, and monolithic
tiles serialize consumers behind the last producer.
LN affine (w,b) is folded into the following weight matrices host-side.
Pool alloc/release is strict LIFO; qkv weight pools are allocated before
the x pool so their DMAs don't wait on LN1 (stack-address overlap).
"""

import numpy as np

import concourse.bass as bass
from concourse import bacc
import concourse.mybir as mybir
import concourse.tile as tile
from concourse.masks import make_identity

F32 = mybir.dt.float32
F32R = mybir.dt.float32r
BF16 = mybir.dt.bfloat16
FP8 = mybir.dt.float8e4
DR = mybir.MatmulPerfMode.DoubleRow
AF = mybir.ActivationFunctionType
ALU = mybir.AluOpType

B, N, D = 8, 1024, 768
HEADS, HD = 12, 64
HID = 4 * D
EPS = 1e-5
SCALE = HD ** -0.5

_CACHE = {}


def build_program(split_waits=True):
    key = ("nc", split_waits)
    if key in _CACHE:
        return _CACHE[key]

    nc = bacc.Bacc()

    x_h = nc.declare_dram_parameter("x", [N, D], F32, isOutput=False)
    amat_h = nc.declare_dram_parameter("amat", [N, N], F32, isOutput=False)
    wqkvT_h = nc.declare_dram_parameter("wqkvT", [D, 3 * D], FP8, isOutput=False)
    qkvb_h = nc.declare_dram_parameter("qkvb", [3 * D], F32, isOutput=False)
    wprojT_h = nc.declare_dram_parameter("wprojT", [D, D], BF16, isOutput=False)
    bproj_h = nc.declare_dram_parameter("bproj", [D], F32, isOutput=False)
    wfc1T_h = nc.declare_dram_parameter("wfc1T", [D, HID], BF16, isOutput=False)
    fc1b_h = nc.declare_dram_parameter("fc1b", [HID], F32, isOutput=False)
    wfc2T_h = nc.declare_dram_parameter("wfc2T", [HID, D], BF16, isOutput=False)
    bfc2_h = nc.declare_dram_parameter("bfc2", [D], F32, isOutput=False)
    cident_h = nc.declare_dram_parameter("cident", [128, 128], F32, isOutput=False)
    cones_h = nc.declare_dram_parameter("cones", [128], F32, isOutput=False)
    out_h = nc.declare_dram_parameter("out", [N, D], F32, isOutput=True)

    def bcast128(src_ap):
        # [n] dram vector -> [128, n] broadcast access pattern
        return bass.AP(
            tensor=src_ap.tensor,
            offset=src_ap.offset,
            ap=[[0, 128]] + [list(p) for p in src_ap.ap],
        )

    with tile.TileContext(nc) as tc:
        # ---- psum pools (live whole kernel; 4+2+2 = 8 banks) ----
        psum_mm = tc.alloc_tile_pool(name="psmm", bufs=2, space="PSUM")
        psum_tp = tc.alloc_tile_pool(name="pstp", bufs=2, space="PSUM")
        psum_pv = tc.alloc_tile_pool(name="pspv", bufs=2, space="PSUM")

        # ---- constants (live whole kernel) ----
        consts = tc.alloc_tile_pool(name="consts", bufs=1)
        ident = consts.tile([128, 128], F32, name="ident")
        make_identity(nc, ident)
        ident_bf = consts.tile([128, 128], BF16, name="ident_bf")
        make_identity(nc, ident_bf)
        eps_sb = consts.tile([128, 1], F32, name="eps_sb")
        nc.vector.memset(eps_sb, EPS)
        ident_r = consts.tile([128, 128], F32R, name="ident_r")
        ones_row = consts.tile([1, 128], F32R, name="ones_row")
        bproj_row = consts.tile([1, D], F32R, name="bproj_row")
        qkb_sb = consts.tile([128, 12], F32, name="qkb_sb")
        fc1b_sb = consts.tile([128, 24], F32, name="fc1b_sb")
        vbias_bc = consts.tile([128, D], F32, name="vbias_bc")
        bfc2_bc = consts.tile([128, D], F32, name="bfc2_bc")

        # ---- long-lived pools, allocated in lifetime order (LIFO stack) ----
        p_x2 = tc.alloc_tile_pool(name="p_x2", bufs=8)  # proj -> end
        x2ts = [p_x2.tile([128, D], F32R, name=f"x2_{i}", tag="x2") for i in range(8)]
        p_st = tc.alloc_tile_pool(name="p_st", bufs=4)  # LN scratch, reused by LN2
        p_OT = tc.alloc_tile_pool(name="p_OT", bufs=1)  # attention -> proj
        OT = p_OT.tile([128, 6, N], BF16, name="OT")
        p_qk = tc.alloc_tile_pool(name="p_qk", bufs=1)  # qkv -> attention
        qT = p_qk.tile([128, 6, N], BF16, name="qT")
        # kTe: even head rows (0:64) live, odd rows zero; kTo: the reverse.
        # Lets S matmuls run K=128 full-array: the zero half annihilates the
        # other head's q rows.
        kTe = p_qk.tile([128, 6, N], BF16, name="kTe")
        kTo = p_qk.tile([128, 6, N], BF16, name="kTo")
        p_V = tc.alloc_tile_pool(name="p_V", bufs=1)
        V_sb = p_V.tile([128, 8, D], BF16, name="V_sb")
        p_hT = tc.alloc_tile_pool(name="p_hT", bufs=6)  # LN1 -> qkv
        hTq = [
            [p_hT.tile([128, 2, 512], FP8, name=f"hT{dp}{h}") for h in range(2)]
            for dp in range(3)
        ]

        def layer_norm(src_of, dst_of, tiles=range(8)):
            # src_of/dst_of: it -> [128, D] view; dst = (src - mean) * rstd
            for it in tiles:
                src = src_of(it)
                stats = p_st.tile([128, 3, 6], F32, name="stats", tag="stats")
                for sg in range(3):
                    nc.vector.bn_stats(
                        out=stats[:, sg, :],
                        in_=src[:, sg * 256 : (sg + 1) * 256],
                    )
                mv = p_st.tile([128, 2], F32, name="mv", tag="mv")
                nc.vector.bn_aggr(out=mv, in_=stats)
                rstd = p_st.tile([128, 1], F32, name="rstd", tag="rstd")
                nc.scalar.activation(
                    out=rstd, in_=mv[:, 1:2], func=AF.Sqrt, bias=eps_sb
                )
                nc.vector.reciprocal(rstd, rstd)
                nc.vector.tensor_scalar(
                    dst_of(it),
                    src,
                    mv[:, 0:1],
                    rstd,
                    ALU.subtract,
                    ALU.mult,
                )

        def transpose_8xD_to_T(src_of, dst_of, ic4s=(0, 1)):
            # src_of: it -> [128, D] token-major view; dst_of(dt, ic4) -> the
            # [128, 512] feature-major destination slice
            for ic4 in ic4s:
                for dt in range(6):
                    ps = psum_tp.tile([128, 512], F32, name="psT", tag="tp")
                    for k in range(4):
                        nc.tensor.matmul(
                            ps[:, k * 128 : (k + 1) * 128],
                            lhsT=src_of(ic4 * 4 + k)[:, dt * 128 : (dt + 1) * 128],
                            rhs=ident,
                            is_transpose=True,
                            start=(k == 0),
                            stop=(k == 3),
                        )
                    nc.scalar.copy(dst_of(dt, ic4), ps)

        # ================= LN1 (in place over x) =================
        # qkv weight pools allocated before p_x: their SBUF space must not
        # overlap the x tiles, else the weight DMAs wait for LN1 to finish.
        p_wq = tc.alloc_tile_pool(name="p_wq", bufs=12)
        p_wv = tc.alloc_tile_pool(name="p_wv", bufs=2)
        # x arrives as 8 separate tiles (per-tile dependency tracking: LN of
        # tile i starts as soon as its own DMA lands) on two DMA queues,
        # emitted before the constant loads so LN1 is never queued behind
        # them.
        p_x = tc.alloc_tile_pool(name="p_x", bufs=8)
        xts = []
        for it in range(8):
            xt = p_x.tile([128, D], F32, name=f"x{it}", tag="x")
            xts.append(xt)
            q = nc.sync if it % 2 == 0 else nc.gpsimd
            q.dma_start(out=xt, in_=x_h[it * 128 : (it + 1) * 128, :])
        nc.sync.dma_start(out=ident_r, in_=cident_h[:, :].bitcast(F32R))
        nc.sync.dma_start(out=ones_row, in_=cones_h[:].unsqueeze(0).bitcast(F32R))
        nc.sync.dma_start(out=bproj_row, in_=bproj_h[:].unsqueeze(0).bitcast(F32R))
        nc.gpsimd.dma_start(
            out=qkb_sb, in_=qkvb_h[0 : 2 * D].rearrange("(t p) -> p t", p=128)
        )
        nc.gpsimd.dma_start(
            out=fc1b_sb, in_=fc1b_h[:].rearrange("(t p) -> p t", p=128)
        )
        # PE warm-up: full-array (K=128, M=128) f32r matmuls so the HAM
        # clock-gate reaches 8/8 before the LN1 transposes start. Rank-1
        # matmuls do NOT work here (1 of 128 rows busy -> no activity seen).
        # x2ts[6] is an early sync-queue DMA and isn't written until proj.
        warm_ps = psum_pv.tile([128, 512], F32, name="warm", tag="pv")
        for _ in range(12):
            nc.tensor.matmul(
                warm_ps,
                lhsT=ident_r,
                rhs=x2ts[6][:, 0:512],
                start=True,
                stop=True,
            )
        ln1_tp = lambda i4: transpose_8xD_to_T(
            lambda it: xts[it],
            lambda dt, _i4: hTq[dt // 2][_i4][:, dt % 2, :],
            ic4s=(i4,),
        )
        layer_norm(lambda it: xts[it], lambda it: xts[it], tiles=range(0, 4))
        ln1_tp(0)
        layer_norm(lambda it: xts[it], lambda it: xts[it], tiles=range(4, 8))

        # ================= QKV =================
        # (note: LN1's second transpose half is emitted in the middle of the
        # q/k loop below, so the PE works on qkv tcn=0 while LN1 finishes)
        nc.gpsimd.memset(kTe[64:128, :, :], 0.0)
        nc.gpsimd.memset(kTo[0:64, :, :], 0.0)
        wqs = []
        for ft in range(12):
            wq = p_wq.tile([128, 6, 128], FP8, name="wq", tag="wq")
            wqs.append(wq)
            nc.gpsimd.dma_start(
                out=wq,
                in_=wqkvT_h[:, ft * 128 : (ft + 1) * 128].rearrange(
                    "(t p) f -> p t f", p=128
                ),
            )
        for tcn in range(2):
            if tcn == 1:
                # PE queue: LN1's ic4=1 transposes land after the tcn=0
                # matmuls (their hTq[..][0] inputs were ready much earlier)
                ln1_tp(1)
                p_x.release()
            for ft in range(12):
                wq = wqs[ft]
                ps = psum_mm.tile([128, 1024], F32, name="psq", tag="mm")
                for dp in range(3):
                    nc.tensor.matmul(
                        ps[:, 0:512],
                        lhsT=wq[:, 2 * dp : 2 * dp + 2, :],
                        rhs=hTq[dp][tcn],
                        start=(dp == 0),
                        stop=(dp == 2),
                        perf_mode=DR,
                    )
                sl = slice(tcn * 512, (tcn + 1) * 512)
                if ft < 6:
                    nc.scalar.activation(
                        out=qT[:, ft, sl], in_=ps[:, 0:512],
                        func=AF.Identity, bias=qkb_sb[:, ft : ft + 1],
                    )
                else:
                    col = ft - 6
                    nc.scalar.activation(
                        out=kTe[0:64, col, sl], in_=ps[0:64, 0:512],
                        func=AF.Identity, bias=qkb_sb[0:64, ft : ft + 1],
                    )
                    nc.vector.tensor_scalar(
                        kTo[64:128, col, sl],
                        ps[64:128, 0:512],
                        qkb_sb[64:128, ft : ft + 1],
                        None,
                        ALU.add,
                    )

        nc.gpsimd.dma_start(out=vbias_bc, in_=bcast128(qkvb_h[2 * D : 3 * D]))
        for f0, fw in ((0, 512), (512, 256)):
            wv = p_wv.tile([128, 6, 512], FP8, name="wv", tag="wv")
            nc.gpsimd.dma_start(
                out=wv[:, :, 0:fw],
                in_=wqkvT_h[:, 2 * D + f0 : 2 * D + f0 + fw].rearrange(
                    "(t p) f -> p t f", p=128
                ),
            )
            for it in range(8):
                ps = psum_mm.tile([128, 1024], F32, name="psv", tag="mm")
                for dp in range(3):
                    nc.tensor.matmul(
                        ps[:, 0:fw],
                        lhsT=hTq[dp][it // 4][
                            :, :, (it % 4) * 128 : (it % 4 + 1) * 128
                        ],
                        rhs=wv[:, 2 * dp : 2 * dp + 2, 0:fw],
                        start=(dp == 0),
                        stop=(dp == 2),
                        perf_mode=DR,
                    )
                nc.vector.tensor_add(
                    V_sb[:, it, f0 : f0 + fw], ps[:, 0:fw], vbias_bc[:, f0 : f0 + fw]
                )

        p_wv.release()
        p_wq.release()
        p_hT.release()

        # ================= attention =================
        p_wp = tc.alloc_tile_pool(name="p_wp", bufs=1)
        wproj = p_wp.tile([128, 6, D], BF16, name="wproj")
        nc.gpsimd.dma_start(
            out=wproj, in_=wprojT_h[:, :].rearrange("(t p) f -> p t f", p=128)
        )
        p_am = tc.alloc_tile_pool(name="p_am", bufs=2)
        p_e = tc.alloc_tile_pool(name="p_e", bufs=5)
        p_PT = tc.alloc_tile_pool(name="p_PT", bufs=4)
        p_dn = tc.alloc_tile_pool(name="p_dn", bufs=2)

        am_tiles = {}

        def load_am(ic):
            am = p_am.tile([128, 4, N], BF16, name="am", tag="am")
            nc.gpsimd.dma_start(
                out=am,
                in_=amat_h[ic * 512 : (ic + 1) * 512, :].rearrange(
                    "(t p) j -> p t j", p=128
                ),
            )
            am_tiles[ic] = am

        def stage_a(ic, hp):
            # S = q^T k row-major (K=128 via zero-padded k), exp + denom
            e0 = p_e.tile([128, 4, N], BF16, name="e0", tag="e")
            e1 = p_e.tile([128, 4, N], BF16, name="e1", tag="e")
            dens = p_dn.tile([128, 8], F32, name="dens", tag="dens")
            for it2 in range(4):
                isl = slice(ic * 512 + it2 * 128, ic * 512 + (it2 + 1) * 128)
                for e_h, kTz, c0 in ((e0, kTe, 0), (e1, kTo, 4)):
                    ps = psum_mm.tile([128, 1024], F32, name="psS", tag="mm")
                    for jc in range(2):
                        nc.tensor.matmul(
                            ps[:, jc * 512 : (jc + 1) * 512],
                            lhsT=qT[:, hp, isl],
                            rhs=kTz[:, hp, jc * 512 : (jc + 1) * 512],
                            start=True,
                            stop=True,
                        )
                    nc.scalar.activation(
                        out=e_h[:, it2, :],
                        in_=ps,
                        func=AF.Exp,
                        scale=SCALE / 256.0,
                        accum_out=dens[:, c0 + it2 : c0 + it2 + 1],
                    )
            return e0, e1, dens

        def stage_b(ic, hp, e0, e1, dens):
            h0, h1 = 2 * hp, 2 * hp + 1
            am = am_tiles[ic]
            rden = p_dn.tile([128, 8], F32, name="rden", tag="rden")
            nc.vector.reciprocal(rden, dens)
            for it2 in range(4):
                for e_h, c0 in ((e0, 0), (e1, 4)):
                    # 4x-mode tensor_scalar (per-it2: rden is a per-partition
                    # scalar that differs per q-tile)
                    nc.vector.tensor_scalar(
                        e_h[:, it2, :],
                        e_h[:, it2, :],
                        rden[:, c0 + it2 : c0 + it2 + 1],
                        None,
                        ALU.mult,
                    )
            for e_h in (e0, e1):
                # one 2x-mode tensor_tensor over the whole [128, 4096] tile
                # (amortizes the per-op overhead 4x)
                nc.vector.tensor_tensor(
                    out=e_h[:, :, :], in0=e_h[:, :, :], in1=am[:, :, :],
                    op=ALU.add,
                )
            PTs = []
            for ei, e_h in enumerate((e0, e1)):
                PT = p_PT.tile([128, 8, 512], BF16, name="PT", tag="PT")
                PTs.append(PT)
                for jp in range(4):  # two k-tiles per bf16 psum bank
                    ps = psum_tp.tile([128, 1024], BF16, name="psP", tag="tp")
                    for j2 in range(2):
                        jt = 2 * jp + j2
                        for k in range(4):
                            nc.tensor.matmul(
                                ps[:, j2 * 512 + k * 128 : j2 * 512 + (k + 1) * 128],
                                lhsT=e_h[:, k, jt * 128 : (jt + 1) * 128],
                                rhs=ident_bf,
                                is_transpose=True,
                                start=(j2 == 0 and k == 0),
                                stop=(j2 == 1 and k == 3),
                            )
                    if jp == 1 + 2 * ei:  # one pair per head on ACT
                        nc.scalar.activation(
                            out=PT[:, 2 * jp : 2 * jp + 2, :], in_=ps, func=AF.Relu
                        )
                    else:
                        nc.vector.tensor_scalar(
                            PT[:, 2 * jp : 2 * jp + 2, :], ps, 0.0, 1.0,
                            ALU.max, ALU.min,
                        )
            po = psum_pv.tile([128, 512], F32, name="po", tag="pv")
            for jt in range(8):
                nc.tensor.matmul(
                    po[0:64, :],
                    lhsT=V_sb[:, jt, h0 * 64 : (h0 + 1) * 64],
                    rhs=PTs[0][:, jt, :],
                    start=(jt == 0),
                    stop=(jt == 7),
                    tile_position=(0, 0),
                )
                nc.tensor.matmul(
                    po[64:128, :],
                    lhsT=V_sb[:, jt, h1 * 64 : (h1 + 1) * 64],
                    rhs=PTs[1][:, jt, :],
                    start=(jt == 0),
                    stop=(jt == 7),
                    tile_position=(0, 64),
                    skip_group_check=True,
                )
            nc.scalar.copy(OT[:, hp, ic * 512 : (ic + 1) * 512], po)

        steps = [(ic, hp) for ic in range(2) for hp in range(6)]
        load_am(0)
        pending = None
        for idx, (ic, hp) in enumerate(steps):
            if hp == 0 and ic + 1 < 2:
                load_am(ic + 1)
            staged = stage_a(ic, hp)
            if pending is not None:
                stage_b(*pending)
            pending = (ic, hp, *staged)
        stage_b(*pending)

        p_dn.release()
        p_PT.release()
        p_e.release()
        p_am.release()

        # ================= proj + residual -> x2 =================
        # residual x and bias ride the PE accumulation (identity / rank-1
        # matmuls); epilogue is a single ACT copy, keeping DVE free for LN2.
        for it in range(8):
            q = nc.sync if it % 2 == 0 else nc.gpsimd
            q.dma_start(
                out=x2ts[it],
                in_=x_h[it * 128 : (it + 1) * 128, :].bitcast(F32R),
            )
        for it in range(8):
            for f0, fw in ((0, 512), (512, 256)):
                ps = psum_mm.tile([128, 1024], F32, name="psp", tag="mm")
                for dt in range(6):
                    nc.tensor.matmul(
                        ps[:, 0:fw],
                        lhsT=(OT[:, dt, it * 128 : (it + 1) * 128]),
                        rhs=(wproj[:, dt, f0 : f0 + fw]),
                        start=(dt == 0),
                        stop=False,
                    )
                nc.tensor.matmul(
                    ps[:, 0:fw],
                    lhsT=ident_r,
                    rhs=x2ts[it][:, f0 : f0 + fw],
                    start=False,
                    stop=False,
                )
                nc.tensor.matmul(
                    ps[:, 0:fw],
                    lhsT=ones_row,
                    rhs=bproj_row[:, f0 : f0 + fw],
                    start=False,
                    stop=True,
                )
                nc.scalar.copy(x2ts[it][:, f0 : f0 + fw], ps[:, 0:fw])
        p_wp.release()
        p_V.release()
        p_qk.release()
        p_OT.release()

        # ================= LN2 =================
        p_h2T = tc.alloc_tile_pool(name="p_h2T", bufs=2)
        h2Th = [p_h2T.tile([128, 6, 512], BF16, name=f"h2T{h}") for h in range(2)]
        p_w1 = tc.alloc_tile_pool(name="p_w1", bufs=2)
        p_a1 = tc.alloc_tile_pool(name="p_a1", bufs=2)
        p_w2 = tc.alloc_tile_pool(name="p_w2", bufs=2)
        p_h2 = tc.alloc_tile_pool(name="p_h2", bufs=1)
        h2_sb = p_h2.tile([128, 8, D], F32, name="h2_sb")
        for ic4 in range(2):
            layer_norm(lambda it: x2ts[it], lambda it: h2_sb[:, it, :],
                       tiles=range(ic4 * 4, ic4 * 4 + 4))
            transpose_8xD_to_T(lambda it: h2_sb[:, it, :],
                               lambda dt, i4: h2Th[i4][:, dt, :],
                               ic4s=(ic4,))
        p_h2.release()

        # ============ MLP (hidden-chunked, accumulate into x2) ============
        for hc in range(4):
            w1 = p_w1.tile([128, 6, 6, 128], BF16, name="w1", tag="w1")
            nc.gpsimd.dma_start(
                out=w1,
                in_=wfc1T_h[:, hc * 768 : (hc + 1) * 768].rearrange(
                    "(t p) (s f) -> p t s f", p=128, f=128
                ),
            )
            a1h = [
                p_a1.tile([128, 6, 512], BF16, name=f"a1{h}", tag="a1")
                for h in range(2)
            ]
            for tcn in range(2):
                for hti in range(6):
                    ht = hc * 6 + hti
                    ps = psum_mm.tile([128, 1024], F32, name="ps1", tag="mm")
                    for dt in range(6):
                        nc.tensor.matmul(
                            ps[:, 0:512],
                            lhsT=(w1[:, dt, hti, :]),
                            rhs=(h2Th[tcn][:, dt, :]),
                            start=(dt == 0),
                            stop=(dt == 5),
                        )
                    nc.scalar.activation(
                        out=a1h[tcn][:, hti, :],
                        in_=ps[:, 0:512],
                        func=AF.Gelu,
                        bias=fc1b_sb[:, ht : ht + 1],
                    )
            for dc in range(3):
                w2 = p_w2.tile([128, 6, 256], BF16, name="w2", tag="w2")
                nc.gpsimd.dma_start(
                    out=w2,
                    in_=wfc2T_h[
                        hc * 768 : (hc + 1) * 768, dc * 256 : (dc + 1) * 256
                    ].rearrange("(t p) f -> p t f", p=128),
                )
                for it in range(8):
                    ps = psum_tp.tile([128, 512], F32, name="ps2", tag="tp")
                    for hti in range(6):
                        nc.tensor.matmul(
                            ps[:, 0:256],
                            lhsT=(
                                a1h[it // 4][
                                    :, hti, (it % 4) * 128 : (it % 4 + 1) * 128
                                ]
                            ),
                            rhs=(w2[:, hti, :]),
                            start=(hti == 0),
                            stop=(hti == 5),
                        )
                    sl = x2ts[it][:, dc * 256 : (dc + 1) * 256]
                    nc.vector.tensor_add(sl, ps[:, 0:256], sl)

        p_w2.release()
        p_a1.release()
        p_w1.release()
        p_h2T.release()

        # ================= final bias + store =================
        nc.gpsimd.dma_start(out=bfc2_bc, in_=bcast128(bfc2_h[:]))
        for it in range(8):
            nc.vector.tensor_add(x2ts[it], x2ts[it], bfc2_bc)
            nc.sync.dma_start(
                out=out_h[it * 128 : (it + 1) * 128, :].bitcast(F32R),
                in_=x2ts[it],
            )

        p_st.release()
        p_x2.release()
        consts.release()
        psum_pv.release()
        psum_tp.release()
        psum_mm.release()

    if split_waits:
        nc.compile()
    _CACHE[key] = nc
    return nc


def _split_matmul_waits(nc, max_mm_waits=1, chunk=4):
    """walrus's Matmult S3_LW struct supports very few semaphore waits; move
    a multi-wait matmul's waits onto PE NoOps inserted just before it (PE
    executes in order, so the waits still gate the matmul)."""
    n_split = 0
    for fn in nc.m.functions:
        for bb in fn.blocks:
            new = []
            for inst in bb.instructions:
                si = inst.sync_info
                if (
                    type(inst).__name__ == "InstMatmult"
                    and si is not None
                    and len(si.on_wait) > max_mm_waits
                ):
                    waits = list(si.on_wait)
                    for ci in range(0, len(waits), chunk):
                        nop = mybir.InstNoOp(
                            name=f"{inst.name}-w{ci}", ins=[], outs=[]
                        )
                        nop.engine = inst.engine
                        nop.sync_info = mybir.SyncInfo(
                            on_wait=waits[ci : ci + chunk], on_update=[]
                        )
                        new.append(nop)
                    inst.sync_info = mybir.SyncInfo(
                        on_wait=[], on_update=list(si.on_update)
                    )
                    n_split += 1
                new.append(inst)
            bb.instructions = new
    return n_split


def make_in_maps(inputs):
    f = lambda a: np.ascontiguousarray(np.asarray(a, dtype=np.float32))
    x = f(inputs["x"])
    amat = f(inputs["additional_matrix"])
    w_qkv = f(inputs["w_qkv"])
    ln1_w, ln1_b = f(inputs["ln1_w"]), f(inputs["ln1_b"])
    ln2_w, ln2_b = f(inputs["ln2_w"]), f(inputs["ln2_b"])
    w_fc1, b_fc1 = f(inputs["w_fc1"]), f(inputs["b_fc1"])

    import ml_dtypes

    bf = lambda a: np.ascontiguousarray(a.astype(ml_dtypes.bfloat16))
    import ml_dtypes as mld

    f8 = lambda a: np.ascontiguousarray(
        np.clip(a, -240.0, 240.0).astype(mld.float8_e4m3)
    )
    # qkv weights/bias are scaled x16 (dodges fp8e4 subnormals); q,k carry
    # x16 each so exp uses scale/256; V's x16 is folded into wprojT (/16).
    shared = {
        "wqkvT": f8(16.0 * ln1_w[:, None] * w_qkv.T),
        "qkvb": np.ascontiguousarray(16.0 * (ln1_b @ w_qkv.T)),
        "wprojT": bf(f(inputs["w_proj"]).T / 16.0),
        "bproj": f(inputs["b_proj"]),
        "wfc1T": bf(ln2_w[:, None] * w_fc1.T),
        "fc1b": np.ascontiguousarray(b_fc1 + ln2_b @ w_fc1.T),
        "wfc2T": bf(f(inputs["w_fc2"]).T),
        "bfc2": f(inputs["b_fc2"]),
        "cident": np.eye(128, dtype=np.float32),
        "cones": np.ones(128, dtype=np.float32),
    }
    return [
        {"x": np.ascontiguousarray(x[b]), "amat": np.ascontiguousarray(amat[b, 0]), **shared}
        for b in range(B)
    ]


def kernel(**inputs) -> np.ndarray:
    from concourse.bass_utils import run_bass_kernel_spmd

    nc = build_program()
    in_maps = make_in_maps(inputs)
    res = run_bass_kernel_spmd(nc, in_maps, list(range(B)))
    return np.stack([res.results[b]["out"] for b in range(B)]).astype(np.float32)



# revision 36
# speedup vs baseline: 1.0057x; 1.0016x over previous
"""Trainium2 Bass kernel for a dense transformer block.

Problem: nn_Block (B=8, N=1024, D=768, H=12, HID=3072), fp32.
Sharding: data-parallel over batch, one batch element per NeuronCore (8 cores).

Per-core program (all in one TileContext):
  LN1 (per-tile x, dual DMA queues) -> PE-transpose -> qkv in fp8e4
  DoubleRow (weights x16 host-side; 1/256 folded into the exp scale, V's
  x16 into wprojT/16); q,k feature-major, V token-major.
  attention per (i_chunk, head-pair): S=q@kT row-major (K=128 via
  zero-padded k), exp(+accum denom) on ACT, normalize (TS 4x) + additive
  bias (one full-tile TT 2x) on DVE, PE-transpose pairs into single bf16
  PSUM banks, clamp[0,1] on the PSUM->SBUF copy, P^T @ V -> O^T.
  proj: residual x rides the PE accumulation as a float32r identity
  matmul, bias as a K=1 rank-1 matmul, ACT copy-out (DVE stays free so
  LN2 overlaps proj). LN2 -> transpose, MLP bf16 hidden-chunked
  accumulating into x2, final bias add -> out.

Big SBUF tensors are split per consumer granularity (x 8 tiles, hT 6,
h2T/a1 halves): Tile dependency tracking is per-tile, and monolithic
tiles serialize consumers behind the last producer.
LN affine (w,b) is folded into the following weight matrices host-side.
Pool alloc/release is strict LIFO; qkv weight pools are allocated before
the x pool so their DMAs don't wait on LN1 (stack-address overlap).
"""

import numpy as np

import concourse.bass as bass
from concourse import bacc
import concourse.mybir as mybir
import concourse.tile as tile
from concourse.masks import make_identity

F32 = mybir.dt.float32
F32R = mybir.dt.float32r
BF16 = mybir.dt.bfloat16
FP8 = mybir.dt.float8e4
DR = mybir.MatmulPerfMode.DoubleRow
AF = mybir.ActivationFunctionType
ALU = mybir.AluOpType

B, N, D = 8, 1024, 768
HEADS, HD = 12, 64
HID = 4 * D
EPS = 1e-5
SCALE = HD ** -0.5

_CACHE = {}


def build_program(split_waits=True):
    key = ("nc", split_waits)
    if key in _CACHE:
        return _CACHE[key]

    nc = bacc.Bacc()

    x_h = nc.declare_dram_parameter("x", [N, D], F32, isOutput=False)
    amat_h = nc.declare_dram_parameter("amat", [N, N], F32, isOutput=False)
    wqkvT_h = nc.declare_dram_parameter("wqkvT", [D, 3 * D], FP8, isOutput=False)
    qkvb_h = nc.declare_dram_parameter("qkvb", [3 * D], F32, isOutput=False)
    wprojT_h = nc.declare_dram_parameter("wprojT", [D, D], BF16, isOutput=False)
    bproj_h = nc.declare_dram_parameter("bproj", [D], F32, isOutput=False)
    wfc1T_h = nc.declare_dram_parameter("wfc1T", [D, HID], BF16, isOutput=False)
    fc1b_h = nc.declare_dram_parameter("fc1b", [HID], F32, isOutput=False)
    wfc2T_h = nc.declare_dram_parameter("wfc2T", [HID, D], BF16, isOutput=False)
    bfc2_h = nc.declare_dram_parameter("bfc2", [D], F32, isOutput=False)
    cident_h = nc.declare_dram_parameter("cident", [128, 128], F32, isOutput=False)
    cones_h = nc.declare_dram_parameter("cones", [128], F32, isOutput=False)
    out_h = nc.declare_dram_parameter("out", [N, D], F32, isOutput=True)

    def bcast128(src_ap):
        # [n] dram vector -> [128, n] broadcast access pattern
        return bass.AP(
            tensor=src_ap.tensor,
            offset=src_ap.offset,
            ap=[[0, 128]] + [list(p) for p in src_ap.ap],
        )

    with tile.TileContext(nc) as tc:
        # ---- psum pools (live whole kernel; 4+2+2 = 8 banks) ----
        psum_mm = tc.alloc_tile_pool(name="psmm", bufs=2, space="PSUM")
        psum_tp = tc.alloc_tile_pool(name="pstp", bufs=2, space="PSUM")
        psum_pv = tc.alloc_tile_pool(name="pspv", bufs=2, space="PSUM")

        # ---- constants (live whole kernel) ----
        consts = tc.alloc_tile_pool(name="consts", bufs=1)
        ident = consts.tile([128, 128], F32, name="ident")
        make_identity(nc, ident)
        ident_bf = consts.tile([128, 128], BF16, name="ident_bf")
        make_identity(nc, ident_bf)
        eps_sb = consts.tile([128, 1], F32, name="eps_sb")
        nc.vector.memset(eps_sb, EPS)
        ident_r = consts.tile([128, 128], F32R, name="ident_r")
        ones_row = consts.tile([1, 128], F32R, name="ones_row")
        bproj_row = consts.tile([1, D], F32R, name="bproj_row")
        qkb_sb = consts.tile([128, 12], F32, name="qkb_sb")
        fc1b_sb = consts.tile([128, 24], F32, name="fc1b_sb")
        vbias_bc = consts.tile([128, D], F32, name="vbias_bc")
        bfc2_bc = consts.tile([128, D], F32, name="bfc2_bc")

        # ---- long-lived pools, allocated in lifetime order (LIFO stack) ----
        p_x2 = tc.alloc_tile_pool(name="p_x2", bufs=8)  # proj -> end
        x2ts = [p_x2.tile([128, D], F32R, name=f"x2_{i}", tag="x2") for i in range(8)]
        p_st = tc.alloc_tile_pool(name="p_st", bufs=4)  # LN scratch, reused by LN2
        p_OT = tc.alloc_tile_pool(name="p_OT", bufs=1)  # attention -> proj
        OT = p_OT.tile([128, 6, N], BF16, name="OT")
        p_qk = tc.alloc_tile_pool(name="p_qk", bufs=1)  # qkv -> attention
        qT = p_qk.tile([128, 6, N], BF16, name="qT")
        # kTe: even head rows (0:64) live, odd rows zero; kTo: the reverse.
        # Lets S matmuls run K=128 full-array: the zero half annihilates the
        # other head's q rows.
        kTe = p_qk.tile([128, 6, N], BF16, name="kTe")
        kTo = p_qk.tile([128, 6, N], BF16, name="kTo")
        p_V = tc.alloc_tile_pool(name="p_V", bufs=1)
        V_sb = p_V.tile([128, 8, D], BF16, name="V_sb")
        p_hT = tc.alloc_tile_pool(name="p_hT", bufs=6)  # LN1 -> qkv
        hTq = [
            [p_hT.tile([128, 2, 512], FP8, name=f"hT{dp}{h}") for h in range(2)]
            for dp in range(3)
        ]

        def layer_norm(src_of, dst_of, tiles=range(8)):
            # src_of/dst_of: it -> [128, D] view; dst = (src - mean) * rstd
            for it in tiles:
                src = src_of(it)
                stats = p_st.tile([128, 3, 6], F32, name="stats", tag="stats")
                for sg in range(3):
                    nc.vector.bn_stats(
                        out=stats[:, sg, :],
                        in_=src[:, sg * 256 : (sg + 1) * 256],
                    )
                mv = p_st.tile([128, 2], F32, name="mv", tag="mv")
                nc.vector.bn_aggr(out=mv, in_=stats)
                rstd = p_st.tile([128, 1], F32, name="rstd", tag="rstd")
                nc.scalar.activation(
                    out=rstd, in_=mv[:, 1:2], func=AF.Sqrt, bias=eps_sb
                )
                nc.vector.reciprocal(rstd, rstd)
                nc.vector.tensor_scalar(
                    dst_of(it),
                    src,
                    mv[:, 0:1],
                    rstd,
                    ALU.subtract,
                    ALU.mult,
                )

        def transpose_8xD_to_T(src_of, dst_of, ic4s=(0, 1)):
            # src_of: it -> [128, D] token-major view; dst_of(dt, ic4) -> the
            # [128, 512] feature-major destination slice
            for ic4 in ic4s:
                for dt in range(6):
                    ps = psum_tp.tile([128, 512], F32, name="psT", tag="tp")
                    for k in range(4):
                        nc.tensor.matmul(
                            ps[:, k * 128 : (k + 1) * 128],
                            lhsT=src_of(ic4 * 4 + k)[:, dt * 128 : (dt + 1) * 128],
                            rhs=ident,
                            is_transpose=True,
                            start=(k == 0),
                            stop=(k == 3),
                        )
                    nc.scalar.copy(dst_of(dt, ic4), ps)

        # ================= LN1 (in place over x) =================
        # qkv weight pools allocated before p_x: their SBUF space must not
        # overlap the x tiles, else the weight DMAs wait for LN1 to finish.
        p_wq = tc.alloc_tile_pool(name="p_wq", bufs=12)
        p_wv = tc.alloc_tile_pool(name="p_wv", bufs=2)
        # x arrives as 8 separate tiles (per-tile dependency tracking: LN of
        # tile i starts as soon as its own DMA lands) on two DMA queues,
        # emitted before the constant loads so LN1 is never queued behind
        # them.
        p_x = tc.alloc_tile_pool(name="p_x", bufs=8)
        xts = []
        for it in range(8):
            xt = p_x.tile([128, D], F32, name=f"x{it}", tag="x")
            xts.append(xt)
            q = nc.sync if it % 2 == 0 else nc.gpsimd
            q.dma_start(out=xt, in_=x_h[it * 128 : (it + 1) * 128, :])
        nc.sync.dma_start(out=ident_r, in_=cident_h[:, :].bitcast(F32R))
        nc.sync.dma_start(out=ones_row, in_=cones_h[:].unsqueeze(0).bitcast(F32R))
        nc.sync.dma_start(out=bproj_row, in_=bproj_h[:].unsqueeze(0).bitcast(F32R))
        nc.gpsimd.dma_start(
            out=qkb_sb, in_=qkvb_h[0 : 2 * D].rearrange("(t p) -> p t", p=128)
        )
        nc.gpsimd.dma_start(
            out=fc1b_sb, in_=fc1b_h[:].rearrange("(t p) -> p t", p=128)
        )
        # PE warm-up: full-array (K=128, M=128) f32r matmuls so the HAM
        # clock-gate reaches 8/8 before the LN1 transposes start. Rank-1
        # matmuls do NOT work here (1 of 128 rows busy -> no activity seen).
        # x2ts[6] is an early sync-queue DMA and isn't written until proj.
        warm_ps = psum_pv.tile([128, 512], F32, name="warm", tag="pv")
        for _ in range(48):
            nc.tensor.matmul(
                warm_ps[:, 0:128],
                lhsT=ident_r,
                rhs=ident_r,
                start=True,
                stop=True,
            )
        ln1_tp = lambda i4: transpose_8xD_to_T(
            lambda it: xts[it],
            lambda dt, _i4: hTq[dt // 2][_i4][:, dt % 2, :],
            ic4s=(i4,),
        )
        layer_norm(lambda it: xts[it], lambda it: xts[it], tiles=range(0, 4))
        ln1_tp(0)
        layer_norm(lambda it: xts[it], lambda it: xts[it], tiles=range(4, 8))

        # ================= QKV =================
        # (note: LN1's second transpose half is emitted in the middle of the
        # q/k loop below, so the PE works on qkv tcn=0 while LN1 finishes)
        nc.gpsimd.memset(kTe[64:128, :, :], 0.0)
        nc.gpsimd.memset(kTo[0:64, :, :], 0.0)
        wqs = []
        for ft in range(12):
            wq = p_wq.tile([128, 6, 128], FP8, name="wq", tag="wq")
            wqs.append(wq)
            nc.gpsimd.dma_start(
                out=wq,
                in_=wqkvT_h[:, ft * 128 : (ft + 1) * 128].rearrange(
                    "(t p) f -> p t f", p=128
                ),
            )
        for tcn in range(2):
            if tcn == 1:
                # PE queue: LN1's ic4=1 transposes land after the tcn=0
                # matmuls (their hTq[..][0] inputs were ready much earlier)
                ln1_tp(1)
                p_x.release()
            for ft in range(12):
                wq = wqs[ft]
                ps = psum_mm.tile([128, 1024], F32, name="psq", tag="mm")
                for dp in range(3):
                    nc.tensor.matmul(
                        ps[:, 0:512],
                        lhsT=wq[:, 2 * dp : 2 * dp + 2, :],
                        rhs=hTq[dp][tcn],
                        start=(dp == 0),
                        stop=(dp == 2),
                        perf_mode=DR,
                    )
                sl = slice(tcn * 512, (tcn + 1) * 512)
                if ft < 6:
                    nc.scalar.activation(
                        out=qT[:, ft, sl], in_=ps[:, 0:512],
                        func=AF.Identity, bias=qkb_sb[:, ft : ft + 1],
                    )
                else:
                    col = ft - 6
                    nc.scalar.activation(
                        out=kTe[0:64, col, sl], in_=ps[0:64, 0:512],
                        func=AF.Identity, bias=qkb_sb[0:64, ft : ft + 1],
                    )
                    nc.vector.tensor_scalar(
                        kTo[64:128, col, sl],
                        ps[64:128, 0:512],
                        qkb_sb[64:128, ft : ft + 1],
                        None,
                        ALU.add,
                    )

        nc.gpsimd.dma_start(out=vbias_bc, in_=bcast128(qkvb_h[2 * D : 3 * D]))
        for f0, fw in ((0, 512), (512, 256)):
            wv = p_wv.tile([128, 6, 512], FP8, name="wv", tag="wv")
            nc.gpsimd.dma_start(
                out=wv[:, :, 0:fw],
                in_=wqkvT_h[:, 2 * D + f0 : 2 * D + f0 + fw].rearrange(
                    "(t p) f -> p t f", p=128
                ),
            )
            for it in range(8):
                ps = psum_mm.tile([128, 1024], F32, name="psv", tag="mm")
                for dp in range(3):
                    nc.tensor.matmul(
                        ps[:, 0:fw],
                        lhsT=hTq[dp][it // 4][
                            :, :, (it % 4) * 128 : (it % 4 + 1) * 128
                        ],
                        rhs=wv[:, 2 * dp : 2 * dp + 2, 0:fw],
                        start=(dp == 0),
                        stop=(dp == 2),
                        perf_mode=DR,
                    )
                nc.vector.tensor_add(
                    V_sb[:, it, f0 : f0 + fw], ps[:, 0:fw], vbias_bc[:, f0 : f0 + fw]
                )

        p_wv.release()
        p_wq.release()
        p_hT.release()

        # ================= attention =================
        p_wp = tc.alloc_tile_pool(name="p_wp", bufs=1)
        wproj = p_wp.tile([128, 6, D], BF16, name="wproj")
        nc.gpsimd.dma_start(
            out=wproj, in_=wprojT_h[:, :].rearrange("(t p) f -> p t f", p=128)
        )
        p_am = tc.alloc_tile_pool(name="p_am", bufs=2)
        p_e = tc.alloc_tile_pool(name="p_e", bufs=5)
        p_PT = tc.alloc_tile_pool(name="p_PT", bufs=4)
        p_dn = tc.alloc_tile_pool(name="p_dn", bufs=2)

        am_tiles = {}

        def load_am(ic):
            am = p_am.tile([128, 4, N], BF16, name="am", tag="am")
            nc.gpsimd.dma_start(
                out=am,
                in_=amat_h[ic * 512 : (ic + 1) * 512, :].rearrange(
                    "(t p) j -> p t j", p=128
                ),
            )
            am_tiles[ic] = am

        def stage_a(ic, hp):
            # S = q^T k row-major (K=128 via zero-padded k), exp + denom
            e0 = p_e.tile([128, 4, N], BF16, name="e0", tag="e")
            e1 = p_e.tile([128, 4, N], BF16, name="e1", tag="e")
            dens = p_dn.tile([128, 8], F32, name="dens", tag="dens")
            for it2 in range(4):
                isl = slice(ic * 512 + it2 * 128, ic * 512 + (it2 + 1) * 128)
                for e_h, kTz, c0 in ((e0, kTe, 0), (e1, kTo, 4)):
                    ps = psum_mm.tile([128, 1024], F32, name="psS", tag="mm")
                    for jc in range(2):
                        nc.tensor.matmul(
                            ps[:, jc * 512 : (jc + 1) * 512],
                            lhsT=qT[:, hp, isl],
                            rhs=kTz[:, hp, jc * 512 : (jc + 1) * 512],
                            start=True,
                            stop=True,
                        )
                    nc.scalar.activation(
                        out=e_h[:, it2, :],
                        in_=ps,
                        func=AF.Exp,
                        scale=SCALE / 256.0,
                        accum_out=dens[:, c0 + it2 : c0 + it2 + 1],
                    )
            return e0, e1, dens

        def stage_b(ic, hp, e0, e1, dens):
            h0, h1 = 2 * hp, 2 * hp + 1
            am = am_tiles[ic]
            rden = p_dn.tile([128, 8], F32, name="rden", tag="rden")
            nc.vector.reciprocal(rden, dens)
            for it2 in range(4):
                for e_h, c0 in ((e0, 0), (e1, 4)):
                    # 4x-mode tensor_scalar (per-it2: rden is a per-partition
                    # scalar that differs per q-tile)
                    nc.vector.tensor_scalar(
                        e_h[:, it2, :],
                        e_h[:, it2, :],
                        rden[:, c0 + it2 : c0 + it2 + 1],
                        None,
                        ALU.mult,
                    )
            for e_h in (e0, e1):
                # one 2x-mode tensor_tensor over the whole [128, 4096] tile
                # (amortizes the per-op overhead 4x)
                nc.vector.tensor_tensor(
                    out=e_h[:, :, :], in0=e_h[:, :, :], in1=am[:, :, :],
                    op=ALU.add,
                )
            PTs = []
            for ei, e_h in enumerate((e0, e1)):
                PT = p_PT.tile([128, 8, 512], BF16, name="PT", tag="PT")
                PTs.append(PT)
                for jp in range(4):  # two k-tiles per bf16 psum bank
                    ps = psum_tp.tile([128, 1024], BF16, name="psP", tag="tp")
                    for j2 in range(2):
                        jt = 2 * jp + j2
                        for k in range(4):
                            nc.tensor.matmul(
                                ps[:, j2 * 512 + k * 128 : j2 * 512 + (k + 1) * 128],
                                lhsT=e_h[:, k, jt * 128 : (jt + 1) * 128],
                                rhs=ident_bf,
                                is_transpose=True,
                                start=(j2 == 0 and k == 0),
                                stop=(j2 == 1 and k == 3),
                            )
                    if jp == 1 + 2 * ei:  # one pair per head on ACT
                        nc.scalar.activation(
                            out=PT[:, 2 * jp : 2 * jp + 2, :], in_=ps, func=AF.Relu
                        )
                    else:
                        nc.vector.tensor_scalar(
                            PT[:, 2 * jp : 2 * jp + 2, :], ps, 0.0, 1.0,
                            ALU.max, ALU.min,
                        )
            po = psum_pv.tile([128, 512], F32, name="po", tag="pv")
            for jt in range(8):
                nc.tensor.matmul(
                    po[0:64, :],
                    lhsT=V_sb[:, jt, h0 * 64 : (h0 + 1) * 64],
                    rhs=PTs[0][:, jt, :],
                    start=(jt == 0),
                    stop=(jt == 7),
                    tile_position=(0, 0),
                )
                nc.tensor.matmul(
                    po[64:128, :],
                    lhsT=V_sb[:, jt, h1 * 64 : (h1 + 1) * 64],
                    rhs=PTs[1][:, jt, :],
                    start=(jt == 0),
                    stop=(jt == 7),
                    tile_position=(0, 64),
                    skip_group_check=True,
                )
            nc.scalar.copy(OT[:, hp, ic * 512 : (ic + 1) * 512], po)

        steps = [(ic, hp) for ic in range(2) for hp in range(6)]
        load_am(0)
        pending = None
        for idx, (ic, hp) in enumerate(steps):
            if hp == 0 and ic + 1 < 2:
                load_am(ic + 1)
            staged = stage_a(ic, hp)
            if pending is not None:
                stage_b(*pending)
            pending = (ic, hp, *staged)
        stage_b(*pending)

        p_dn.release()
        p_PT.release()
        p_e.release()
        p_am.release()

        # ================= proj + residual -> x2 =================
        # residual x and bias ride the PE accumulation (identity / rank-1
        # matmuls); epilogue is a single ACT copy, keeping DVE free for LN2.
        for it in range(8):
            q = nc.sync if it % 2 == 0 else nc.gpsimd
            q.dma_start(
                out=x2ts[it],
                in_=x_h[it * 128 : (it + 1) * 128, :].bitcast(F32R),
            )
        for it in range(8):
            for f0, fw in ((0, 512), (512, 256)):
                ps = psum_mm.tile([128, 1024], F32, name="psp", tag="mm")
                for dt in range(6):
                    nc.tensor.matmul(
                        ps[:, 0:fw],
                        lhsT=(OT[:, dt, it * 128 : (it + 1) * 128]),
                        rhs=(wproj[:, dt, f0 : f0 + fw]),
                        start=(dt == 0),
                        stop=False,
                    )
                nc.tensor.matmul(
                    ps[:, 0:fw],
                    lhsT=ident_r,
                    rhs=x2ts[it][:, f0 : f0 + fw],
                    start=False,
                    stop=False,
                )
                nc.tensor.matmul(
                    ps[:, 0:fw],
                    lhsT=ones_row,
                    rhs=bproj_row[:, f0 : f0 + fw],
                    start=False,
                    stop=True,
                )
                nc.scalar.copy(x2ts[it][:, f0 : f0 + fw], ps[:, 0:fw])
        p_wp.release()
        p_V.release()
        p_qk.release()
        p_OT.release()

        # ================= LN2 =================
        p_h2T = tc.alloc_tile_pool(name="p_h2T", bufs=2)
        h2Th = [p_h2T.tile([128, 6, 512], BF16, name=f"h2T{h}") for h in range(2)]
        p_w1 = tc.alloc_tile_pool(name="p_w1", bufs=2)
        p_a1 = tc.alloc_tile_pool(name="p_a1", bufs=2)
        p_w2 = tc.alloc_tile_pool(name="p_w2", bufs=2)
        p_h2 = tc.alloc_tile_pool(name="p_h2", bufs=1)
        h2_sb = p_h2.tile([128, 8, D], F32, name="h2_sb")
        for ic4 in range(2):
            layer_norm(lambda it: x2ts[it], lambda it: h2_sb[:, it, :],
                       tiles=range(ic4 * 4, ic4 * 4 + 4))
            transpose_8xD_to_T(lambda it: h2_sb[:, it, :],
                               lambda dt, i4: h2Th[i4][:, dt, :],
                               ic4s=(ic4,))
        p_h2.release()

        # ============ MLP (hidden-chunked, accumulate into x2) ============
        for hc in range(4):
            w1 = p_w1.tile([128, 6, 6, 128], BF16, name="w1", tag="w1")
            nc.gpsimd.dma_start(
                out=w1,
                in_=wfc1T_h[:, hc * 768 : (hc + 1) * 768].rearrange(
                    "(t p) (s f) -> p t s f", p=128, f=128
                ),
            )
            a1h = [
                p_a1.tile([128, 6, 512], BF16, name=f"a1{h}", tag="a1")
                for h in range(2)
            ]
            for tcn in range(2):
                for hti in range(6):
                    ht = hc * 6 + hti
                    ps = psum_mm.tile([128, 1024], F32, name="ps1", tag="mm")
                    for dt in range(6):
                        nc.tensor.matmul(
                            ps[:, 0:512],
                            lhsT=(w1[:, dt, hti, :]),
                            rhs=(h2Th[tcn][:, dt, :]),
                            start=(dt == 0),
                            stop=(dt == 5),
                        )
                    nc.scalar.activation(
                        out=a1h[tcn][:, hti, :],
                        in_=ps[:, 0:512],
                        func=AF.Gelu,
                        bias=fc1b_sb[:, ht : ht + 1],
                    )
            for dc in range(3):
                w2 = p_w2.tile([128, 6, 256], BF16, name="w2", tag="w2")
                nc.gpsimd.dma_start(
                    out=w2,
                    in_=wfc2T_h[
                        hc * 768 : (hc + 1) * 768, dc * 256 : (dc + 1) * 256
                    ].rearrange("(t p) f -> p t f", p=128),
                )
                for it in range(8):
                    ps = psum_tp.tile([128, 512], F32, name="ps2", tag="tp")
                    for hti in range(6):
                        nc.tensor.matmul(
                            ps[:, 0:256],
                            lhsT=(
                                a1h[it // 4][
                                    :, hti, (it % 4) * 128 : (it % 4 + 1) * 128
                                ]
                            ),
                            rhs=(w2[:, hti, :]),
                            start=(hti == 0),
                            stop=(hti == 5),
                        )
                    sl = x2ts[it][:, dc * 256 : (dc + 1) * 256]
                    nc.vector.tensor_add(sl, ps[:, 0:256], sl)

        p_w2.release()
        p_a1.release()
        p_w1.release()
        p_h2T.release()

        # ================= final bias + store =================
        nc.gpsimd.dma_start(out=bfc2_bc, in_=bcast128(bfc2_h[:]))
        for it in range(8):
            nc.vector.tensor_add(x2ts[it], x2ts[it], bfc2_bc)
            nc.sync.dma_start(
                out=out_h[it * 128 : (it + 1) * 128, :].bitcast(F32R),
                in_=x2ts[it],
            )

        p_st.release()
        p_x2.release()
        consts.release()
        psum_pv.release()
        psum_tp.release()
        psum_mm.release()

    if split_waits:
        nc.compile()
    _CACHE[key] = nc
    return nc


def _split_matmul_waits(nc, max_mm_waits=1, chunk=4):
    """walrus's Matmult S3_LW struct supports very few semaphore waits; move
    a multi-wait matmul's waits onto PE NoOps inserted just before it (PE
    executes in order, so the waits still gate the matmul)."""
    n_split = 0
    for fn in nc.m.functions:
        for bb in fn.blocks:
            new = []
            for inst in bb.instructions:
                si = inst.sync_info
                if (
                    type(inst).__name__ == "InstMatmult"
                    and si is not None
                    and len(si.on_wait) > max_mm_waits
                ):
                    waits = list(si.on_wait)
                    for ci in range(0, len(waits), chunk):
                        nop = mybir.InstNoOp(
                            name=f"{inst.name}-w{ci}", ins=[], outs=[]
                        )
                        nop.engine = inst.engine
                        nop.sync_info = mybir.SyncInfo(
                            on_wait=waits[ci : ci + chunk], on_update=[]
                        )
                        new.append(nop)
                    inst.sync_info = mybir.SyncInfo(
                        on_wait=[], on_update=list(si.on_update)
                    )
                    n_split += 1
                new.append(inst)
            bb.instructions = new
    return n_split


def make_in_maps(inputs):
    f = lambda a: np.ascontiguousarray(np.asarray(a, dtype=np.float32))
    x = f(inputs["x"])
    amat = f(inputs["additional_matrix"])
    w_qkv = f(inputs["w_qkv"])
    ln1_w, ln1_b = f(inputs["ln1_w"]), f(inputs["ln1_b"])
    ln2_w, ln2_b = f(inputs["ln2_w"]), f(inputs["ln2_b"])
    w_fc1, b_fc1 = f(inputs["w_fc1"]), f(inputs["b_fc1"])

    import ml_dtypes

    bf = lambda a: np.ascontiguousarray(a.astype(ml_dtypes.bfloat16))
    import ml_dtypes as mld

    f8 = lambda a: np.ascontiguousarray(
        np.clip(a, -240.0, 240.0).astype(mld.float8_e4m3)
    )
    # qkv weights/bias are scaled x16 (dodges fp8e4 subnormals); q,k carry
    # x16 each so exp uses scale/256; V's x16 is folded into wprojT (/16).
    shared = {
        "wqkvT": f8(16.0 * ln1_w[:, None] * w_qkv.T),
        "qkvb": np.ascontiguousarray(16.0 * (ln1_b @ w_qkv.T)),
        "wprojT": bf(f(inputs["w_proj"]).T / 16.0),
        "bproj": f(inputs["b_proj"]),
        "wfc1T": bf(ln2_w[:, None] * w_fc1.T),
        "fc1b": np.ascontiguousarray(b_fc1 + ln2_b @ w_fc1.T),
        "wfc2T": bf(f(inputs["w_fc2"]).T),
        "bfc2": f(inputs["b_fc2"]),
        "cident": np.eye(128, dtype=np.float32),
        "cones": np.ones(128, dtype=np.float32),
    }
    return [
        {"x": np.ascontiguousarray(x[b]), "amat": np.ascontiguousarray(amat[b, 0]), **shared}
        for b in range(B)
    ]


def kernel(**inputs) -> np.ndarray:
    from concourse.bass_utils import run_bass_kernel_spmd

    nc = build_program()
    in_maps = make_in_maps(inputs)
    res = run_bass_kernel_spmd(nc, in_maps, list(range(B)))
    return np.stack([res.results[b]["out"] for b in range(B)]).astype(np.float32)

